# revision 25
# baseline (speedup 1.0000x reference)
"""Trainium2 Bass kernel for nn_BlockAttentionResidual (sparse block attention + BitNet-style quantized MLP).

Sharding: sequence-block data parallelism. The block attention is independent per
512-token block, so each of the 8 cores owns 1024 contiguous tokens (2 blocks) of
one batch element and runs the ENTIRE layer on them with zero collectives.
  core c -> batch c//4, tokens [(c%4)*1024, (c%4+1)*1024)

Weight ternarization (per-tensor, input-only) is done once on the host; the
device streams SBUF-layout bf16 ternary weights and only does the token-dependent
work: rmsnorm, int8-grid act quant (exact in bf16), matmuls with fp32 PSUM
accumulation, rope, block-causal softmax.

Scheduling: phase working pools alternate between the left and right SBUF
stacks so consecutive phases never trade the same bytes (a pool allocated over
a just-released zone inherits a dependency on that phase's completion, which
would serialize the pipeline).  Cross-phase activation buffers are per
token-tile (or per head-group x block) tiles in four persistent arenas whose
slots are reused by later phases.  The per-token dequant scale (act amax/127 *
weight mean-abs) folds into the rope cos/sin tables for q/k and into a single
tensor_scalar for v / o / down.
"""

import numpy as np
import ml_dtypes

import concourse.bass as bass
import concourse.mybir as mybir
import concourse.tile as tile
from concourse import bacc
from concourse.bass_utils import run_bass_kernel_spmd

F32 = mybir.dt.float32
BF16 = mybir.dt.bfloat16
F16 = mybir.dt.float16
AX = mybir.AxisListType
OP = mybir.AluOpType
ACTF = mybir.ActivationFunctionType

# model dims
H = 2048
NH = 16
HD = 128
NB = 8
INTER = 4096        # 2*H
EPS = 1e-5
THETA = 10000.0
B, S = 2, 4096
BT = 512            # tokens per attention block
NCORES = 8
R = 1024            # tokens per core
NT = R // 128       # 8 token tiles per core
MAGIC = np.float32(1.5 * 2 ** 23)   # fp32 round-to-nearest-even magic
SCALE_QK = float(HD ** -0.5)


def build_program():
    nc = bacc.Bacc(None, target_bir_lowering=False)

    # ---- I/O ----
    x_in = nc.declare_dram_parameter("x_sh", [R, H], F32, isOutput=False)
    cos_in = nc.declare_dram_parameter("cos4_sh", [NT, 128, 256], F32, isOutput=False)
    sin_in = nc.declare_dram_parameter("sin4_sh", [NT, 128, 256], F32, isOutput=False)
    anw_in = nc.declare_dram_parameter("attn_norm_w", [H], F32, isOutput=False)
    fnw_in = nc.declare_dram_parameter("ffn_norm_w", [H], F32, isOutput=False)
    wqkv_in = nc.declare_dram_parameter("wqkv_r", [24, 128, 16 * 256], BF16, isOutput=False)
    wo_in = nc.declare_dram_parameter("wo_r", [4, 128, 16 * 512], BF16, isOutput=False)
    wup_in = nc.declare_dram_parameter("wup_r", [16, 128, 16 * 512], BF16, isOutput=False)
    wdn_in = nc.declare_dram_parameter("wdn_r", [4, 128, 32 * 512], BF16, isOutput=False)
    dqs_in = nc.declare_dram_parameter("dqs", [128, 4], F32, isOutput=False)
    out_d = nc.declare_dram_parameter("out_sh", [R, H], F32, isOutput=True)

    # ---- internal DRAM scratch ----
    x1_d = nc.dram_tensor("x1_d", [NT, 128, H], F32)
    act_d = nc.dram_tensor("act_d", [NT, 128, INTER], F16)

    with tile.TileContext(nc) as tc:
        perm = tc.alloc_tile_pool(name="perm", bufs=1)
        magic_t = perm.tile([128, 1], F32)
        nc.vector.memset(magic_t[:], float(MAGIC))
        magic_ap = magic_t[:]
        dq_sb = perm.tile([128, 4], F32)
        nc.sync.dma_start(dq_sb[:], dqs_in[:])
        s_all = perm.tile([128, NT], F32)
        dqa_all = perm.tile([128, NT], F32)
        s_c = perm.tile([128, NT], F32)
        dqc_all = perm.tile([128, NT], F32)
        s_a = perm.tile([128, NT], F32)
        dqact_all = perm.tile([128, NT], F32)
        amax_str = perm.tile([128, NT, 16], F32)   # ffn act |max| per (t, strip)
        ssq_str = perm.tile([128, NT, 4], F32)     # x1 sum-sq per (t, col-strip)

        # persistent per-tile arenas (slots reused by consecutive phases)
        P1 = tc.alloc_tile_pool(name="P1", bufs=1)   # xqT_t -> ctx_t -> actq_hi_t
        P4 = tc.alloc_tile_pool(name="P4", bufs=1)   # v_gb  -> actq_lo_t
        P3 = tc.alloc_tile_pool(name="P3", bufs=1)   # kT_gb -> hnT_t
        P2 = tc.alloc_tile_pool(name="P2", bufs=1)   # qT_gb -> ctxqT_t

        xqT_t = [P1.tile([128, 16, 128], BF16, tag=f"p1_{t}", name=f"xq_{t}")
                 for t in range(NT)]

        # persistent PSUM pools: 3 + 2 + 2 + 1 = 8 banks
        mm_ps = tc.alloc_tile_pool(name="mm_ps", bufs=3, space="PSUM")
        at_ps = tc.alloc_tile_pool(name="at_ps", bufs=2, space="PSUM")
        cx_ps = tc.alloc_tile_pool(name="cx_ps", bufs=2, space="PSUM")
        upv_ps = tc.alloc_tile_pool(name="upv_ps", bufs=1, space="PSUM")

        # ------------ n1: attn rmsnorm + act-quant + transpose (LEFT) --------
        with nc.named_scope("n1"), \
             tc.tile_pool(name="npool", bufs=2) as npool, \
             tc.tile_pool(name="nwpool", bufs=1) as nwpool:
            anw_b = nwpool.tile([128, H], F32, tag="normw")
            ap0 = anw_in[:]
            nc.gpsimd.dma_start(out=anw_b[:], in_=bass.AP(
                tensor=ap0.tensor, offset=ap0.offset, ap=[[0, 128]] + list(ap0.ap)))
            for t in range(NT):
                xt = npool.tile([128, H], F32, tag="xt")
                nc.sync.dma_start(xt[:], x_in[t * 128:(t + 1) * 128, :])
                ssq = npool.tile([128, 1], F32, tag="ssq")
                junk = npool.tile([128, H], BF16, tag="xq")
                nc.scalar.activation(junk[:], xt[:], ACTF.Square, accum_out=ssq[:])
                msq = npool.tile([128, 1], F32, tag="msq")
                nc.vector.tensor_scalar(msq[:], ssq[:], 1.0 / H, EPS, OP.mult, OP.add)
                sd = npool.tile([128, 1], F32, tag="sd")
                nc.scalar.activation(sd[:], msq[:], ACTF.Sqrt)
                rstd = npool.tile([128, 1], F32, tag="rstd")
                nc.vector.reciprocal(rstd[:], sd[:])
                # y = x*anw; h = y*rstd never materialized (rstd folds into
                # the amax and the ACT round scale, both per-partition)
                y = npool.tile([128, H], F32, tag="y1")
                nc.vector.tensor_tensor(y[:], xt[:], anw_b[:], OP.mult)
                amax = npool.tile([128, 1], F32, tag="amax1")
                nc.vector.tensor_reduce(amax[:], y[:], AX.X, OP.max,
                                        apply_absolute_value=True)
                amh = npool.tile([128, 1], F32, tag="amh1")
                nc.vector.tensor_scalar(amh[:], amax[:], rstd[:], None, OP.mult)
                amc = npool.tile([128, 1], F32, tag="amc1")
                nc.vector.tensor_scalar_max(amc[:], amh[:], 1e-5)
                rec = npool.tile([128, 1], F32, tag="rec1")
                nc.vector.reciprocal(rec[:], amc[:])
                nc.vector.tensor_scalar_mul(s_all[:, t:t + 1], rec[:], 127.0)
                nc.vector.tensor_scalar(dqa_all[:, t:t + 1], amc[:], dq_sb[:, 0:1],
                                        None, OP.mult)
                sy = npool.tile([128, 1], F32, tag="sy1")
                nc.vector.tensor_scalar(sy[:], rstd[:], s_all[:, t:t + 1], None,
                                        OP.mult)
                mg = npool.tile([128, H], F32, tag="xt")
                nc.scalar.activation(mg[:], y[:], ACTF.Identity, bias=magic_ap,
                                     scale=sy[:])
                xq = npool.tile([128, H], BF16, tag="xq")
                nc.gpsimd.tensor_scalar_sub(xq[:], mg[:], float(MAGIC))
                nc.sync.dma_start_transpose(xqT_t[t][:], xq[:])

        # ------------ qkv matmul + rope/dequant + transpose (RIGHT) ----------
        # strips interleaved (q_g, k_g, v_g) so attention can start early;
        # per-token dequant scale is folded into per-t cos/sin tables for q/k.
        qT_gb = [P2.tile([128, 4, 4, 128], BF16, tag=f"p2_{g * 2 + b}", name=f"qT_{g}{b}")
                 for g in range(4) for b in range(2)]
        kT_gb = [P3.tile([128, 4, 4, 128], BF16, tag=f"p3_{g * 2 + b}", name=f"kT_{g}{b}")
                 for g in range(4) for b in range(2)]
        v_gb = [P4.tile([128, 4, 4, 132], BF16, tag=f"p4_{g * 2 + b}", name=f"v_{g}{b}")
                for g in range(4) for b in range(2)]    # [tok, tt, h, hd+aug]
        with nc.named_scope("qkv"), \
             tc.tile_pool(name="qk256", bufs=2, side="right") as qk256, \
             tc.tile_pool(name="cs256", bufs=2, side="right") as cs256, \
             tc.tile_pool(name="cs1_pool", bufs=1, side="right") as cs1_pool:
            # per-t dequant-scaled bf16 rope tables (4-head replicas)
            cosdq = cs1_pool.tile([128, NT, 4, 64], BF16, tag="cosdq")
            sindq = cs1_pool.tile([128, NT, 4, 64], BF16, tag="sindq")
            for t in range(NT):
                cstage = cs1_pool.tile([128, 4, 64], BF16, tag="cstage")
                nc.gpsimd.dma_start(cstage[:], cos_in[t].rearrange("p (c f) -> p c f", c=4))
                nc.vector.tensor_scalar(cosdq[:, t, :, :], cstage[:],
                                        dqa_all[:, t:t + 1], None, OP.mult)
                sstage = cs1_pool.tile([128, 4, 64], BF16, tag="sstage")
                nc.gpsimd.dma_start(sstage[:], sin_in[t].rearrange("p (c f) -> p c f", c=4))
                nc.vector.tensor_scalar(sindq[:, t, :, :], sstage[:],
                                        dqa_all[:, t:t + 1], None, OP.mult)
            # ones column of the augmented v tiles (softmax denominator)
            for gb in range(8):
                nc.vector.memset(v_gb[gb][:, :, :, 128:129], 1.0)

            def rope_store(pv, nh, t, g, kind):
                """Dequant+rope the [128, nh, 128] psum view and store transposed."""
                dst = qT_gb if kind == 0 else kT_gb
                p1, p2 = pv[:, :, 0:64], pv[:, :, 64:128]
                cosd = cosdq[:, t, 0:nh, :]
                sind = sindq[:, t, 0:nh, :]
                pool = cs256 if nh == 2 else cs512
                t1 = pool.tile([128, nh, 64], F32, tag="rt1")
                t2 = pool.tile([128, nh, 64], F32, tag="rt2")
                rot = pool.tile([128, nh, 128], BF16, tag="rot")
                nc.vector.tensor_tensor(t1[:], p1, cosd, OP.mult)
                nc.vector.tensor_tensor(t2[:], p2, sind, OP.mult)
                nc.gpsimd.tensor_tensor(rot[:, :, 0:64], t1[:], t2[:], OP.subtract)
                nc.vector.tensor_tensor(t1[:], p2, cosd, OP.mult)
                nc.vector.tensor_tensor(t2[:], p1, sind, OP.mult)
                nc.gpsimd.tensor_tensor(rot[:, :, 64:128], t1[:], t2[:], OP.add)
                eng = nc.sync if kind == 0 else nc.scalar
                return eng, dst, rot

            # warmup: all of head-group 0 as 256-col strips (small pools
            # coexist with n1; PE fills n1's idle time).  v first: its dequant
            # needs only one DVE op per tile, so it is not paced by n1's DVE.
            for kind in (2, 0, 1):
                for j in range(2):
                    nn = kind * 8 + j
                    wst = qk256.tile([128, 16, 256], BF16, tag="wq2")
                    nc.scalar.dma_start(
                        wst[:], wqkv_in[nn].rearrange("p (k f) -> p k f", k=16))
                    for t in range(NT):
                        psf = at_ps.tile([128, 512], F32, tag="ps_sc")
                        ps = psf[:, 0:256]
                        for kk in range(16):
                            nc.tensor.matmul(ps, xqT_t[t][:, kk, :], wst[:, kk, :],
                                             start=(kk == 0), stop=(kk == 15))
                        pv = ps.rearrange("p (c f) -> p c f", c=2)
                        if kind == 2:
                            nc.vector.tensor_scalar_mul(
                                v_gb[t // 4][:, t % 4, 2 * j:2 * j + 2, 0:128], pv,
                                dqa_all[:, t:t + 1])
                        else:
                            eng, dst, rot = rope_store(pv, 2, t, 0, kind)
                            eng.dma_start_transpose(
                                dst[t // 4][:, 2 * j:2 * j + 2, t % 4, :],
                                rot[:].rearrange("p c f -> p (c f)"))

            with tc.tile_pool(name="qk512", bufs=2, side="right") as qk512, \
                 tc.tile_pool(name="cs512", bufs=2, side="right") as cs512:
                # remaining strips at full 512 width: g=1..3 q,k,v
                rest = [(kind, g) for g in range(1, 4) for kind in range(3)]
                for kind, g in rest:
                    wst = qk512.tile([128, 16, 512], BF16, tag="wq4")
                    nc.sync.dma_start(
                        wst[:, :, 0:256],
                        wqkv_in[kind * 8 + 2 * g].rearrange("p (k f) -> p k f", k=16))
                    nc.sync.dma_start(
                        wst[:, :, 256:512],
                        wqkv_in[kind * 8 + 2 * g + 1].rearrange("p (k f) -> p k f", k=16))
                    for t in range(NT):
                        ps = mm_ps.tile([128, 512], F32, tag="ps_mm")
                        for kk in range(16):
                            nc.tensor.matmul(ps[:], xqT_t[t][:, kk, :], wst[:, kk, :],
                                             start=(kk == 0), stop=(kk == 15))
                        pv = ps[:].rearrange("p (c f) -> p c f", c=4)
                        if kind == 2:
                            nc.vector.tensor_scalar_mul(
                                v_gb[g * 2 + t // 4][:, t % 4, :, 0:128], pv,
                                dqa_all[:, t:t + 1])
                        else:
                            eng, dst, rot = rope_store(pv, 4, t, g, kind)
                            eng.dma_start_transpose(
                                dst[g * 2 + t // 4][:, :, t % 4, :],
                                rot[:].rearrange("p c f -> p (c f)"))

        # ------------ block attention (LEFT) ------------
        ctx_t = [P1.tile([128, 16, 128], BF16, tag=f"p1_{t}", name=f"ctx_{t}")
                 for t in range(NT)]
        with nc.named_scope("attn"), \
             tc.tile_pool(name="apool", bufs=2) as apool:
            for blk in range(2):
                for h in range(NH):
                    g = h // 4
                    hh = h % 4
                    qt_tile = qT_gb[g * 2 + blk]
                    kt_tile = kT_gb[g * 2 + blk]
                    v_tile = v_gb[g * 2 + blk]
                    expT = [None] * 4
                    for kt in range(4):
                        qn = 512 - kt * 128
                        pss = at_ps.tile([128, 512], F32, tag="ps_sc")
                        nc.tensor.matmul(
                            pss[:, 0:qn],
                            kt_tile[:, hh, kt, :],
                            qt_tile[:, hh, :, :]
                            .rearrange("p c f -> p (c f)")[:, kt * 128:512],
                            start=True, stop=True)
                        ex = apool.tile([128, 512], BF16, tag=f"expT{kt}")
                        nc.scalar.activation(ex[:, 0:qn], pss[:, 0:qn], ACTF.Exp,
                                             scale=SCALE_QK)
                        nc.gpsimd.affine_select(
                            out=ex[:, 0:128], in_=ex[:, 0:128],
                            compare_op=OP.is_ge, fill=0.0,
                            base=0, pattern=[[1, 128]], channel_multiplier=-1)
                        expT[kt] = ex
                    for qt in range(4):
                        psc = cx_ps.tile([128, 132], F32, tag="ps_ctx")
                        for kt in range(qt + 1):
                            nc.tensor.matmul(psc[:, 0:129],
                                             expT[kt][:, (qt - kt) * 128:(qt - kt) * 128 + 128],
                                             v_tile[:, kt, hh, 0:129],
                                             start=(kt == 0), stop=(kt == qt))
                        rl = apool.tile([128, 1], F32, tag="rl")
                        nc.vector.reciprocal(rl[:], psc[:, 128:129])
                        nc.vector.tensor_scalar_mul(ctx_t[blk * 4 + qt][:, h, :],
                                                    psc[:, 0:128], rl[:])

        # ------------ ctx quant + transpose (RIGHT) ------------
        ctxqT_t = [P2.tile([128, 16, 128], BF16, tag=f"p2_{(t % 4) * 2 + t // 4}",
                           name=f"ctxqT_{t}") for t in range(NT)]
        with nc.named_scope("ctxq"), \
             tc.tile_pool(name="cqpool", bufs=2, side="right") as cqpool:
            for t in range(NT):
                src = ctx_t[t][:].rearrange("p c f -> p (c f)")
                amax = cqpool.tile([128, 1], F32, tag="amaxc")
                nc.vector.tensor_reduce(amax[:], src, AX.X, OP.max,
                                        apply_absolute_value=True)
                amc = cqpool.tile([128, 1], F32, tag="amcc")
                nc.vector.tensor_scalar_max(amc[:], amax[:], 1e-5)
                rec = cqpool.tile([128, 1], F32, tag="recc")
                nc.vector.reciprocal(rec[:], amc[:])
                nc.vector.tensor_scalar_mul(s_c[:, t:t + 1], rec[:], 127.0)
                nc.vector.tensor_scalar(dqc_all[:, t:t + 1], amc[:], dq_sb[:, 1:2],
                                        None, OP.mult)
                mg = cqpool.tile([128, H], F32, tag="mgc")
                nc.scalar.activation(mg[:], src, ACTF.Identity, bias=magic_ap,
                                     scale=s_c[:, t:t + 1])
                cq = cqpool.tile([128, H], BF16, tag="cq")
                nc.vector.tensor_scalar_sub(cq[:], mg[:], float(MAGIC))
                nc.sync.dma_start_transpose(ctxqT_t[t][:], cq[:])

        # ------ o matmul + residual -> x1_d, ffn-norm inline (LEFT) ------
        # t-outer with all 4 weight strips resident; per-strip Square-accum
        # gathers the x1 sum-of-squares so the norm needs no extra pass.
        hnT_t = [P3.tile([128, 16, 128], BF16, tag=f"p3_{t}", name=f"hnT_{t}")
                 for t in range(NT)]
        with nc.named_scope("oproj"), \
             tc.tile_pool(name="owpool", bufs=1, side="right") as owpool, \
             tc.tile_pool(name="opool", bufs=3) as opool, \
             tc.tile_pool(name="onpool", bufs=2) as onpool, \
             tc.tile_pool(name="ofwpool", bufs=1) as ofwpool:
            fnw_b = ofwpool.tile([128, H], BF16, tag="normw2")
            ap0 = fnw_in[:]
            nc.gpsimd.dma_start(out=fnw_b[:], in_=bass.AP(
                tensor=ap0.tensor, offset=ap0.offset, ap=[[0, 128]] + list(ap0.ap)))
            for grp in range(2):
                wo_st = []
                for j in range(2):
                    w = owpool.tile([128, 16, 512], BF16, tag=f"wo_{j}")
                    nc.sync.dma_start(w[:], wo_in[grp * 2 + j]
                                      .rearrange("p (k f) -> p k f", k=16))
                    wo_st.append(w)
                for t in range(NT):
                    for j in range(2):
                        nn = grp * 2 + j
                        ps = mm_ps.tile([128, 512], F32, tag="ps_mm")
                        for kk in range(16):
                            nc.tensor.matmul(ps[:], ctxqT_t[t][:, kk, :],
                                             wo_st[j][:, kk, :],
                                             start=(kk == 0), stop=(kk == 15))
                        tmp = opool.tile([128, 512], F32, tag="o_tmp")
                        nc.vector.tensor_scalar_mul(tmp[:], ps[:], dqc_all[:, t:t + 1])
                        # residual: x strip added in-flight by the DMA engine
                        nc.gpsimd.dma_start(
                            out=tmp[:], in_=x_in[t * 128:(t + 1) * 128,
                                                 nn * 512:(nn + 1) * 512],
                            accum_op=OP.add)
                        ojunk = opool.tile([128, 512], BF16, tag="ojunk")
                        nc.scalar.activation(ojunk[:], tmp[:], ACTF.Square,
                                             accum_out=ssq_str[:, t, nn:nn + 1])
                        nc.scalar.dma_start(x1_d[t, :, nn * 512:(nn + 1) * 512],
                                            tmp[:])
                    if grp == 1:
                        # ffn rmsnorm for tile t (x1 reloaded once, bf16 path)
                        ssq = onpool.tile([128, 1], F32, tag="ssq2")
                        nc.vector.tensor_reduce(ssq[:], ssq_str[:, t, :], AX.X, OP.add)
                        msq = onpool.tile([128, 1], F32, tag="msq2")
                        nc.vector.tensor_scalar(msq[:], ssq[:], 1.0 / H, EPS,
                                                OP.mult, OP.add)
                        sd = onpool.tile([128, 1], F32, tag="sd2")
                        nc.scalar.activation(sd[:], msq[:], ACTF.Sqrt)
                        rstd = onpool.tile([128, 1], F32, tag="rstd2")
                        nc.vector.reciprocal(rstd[:], sd[:])
                        x1r = onpool.tile([128, H], BF16, tag="x1r")
                        nc.gpsimd.dma_start(x1r[:], x1_d[t, :, :])
                        y2 = onpool.tile([128, H], BF16, tag="y2")
                        nc.vector.tensor_tensor(y2[:], x1r[:], fnw_b[:], OP.mult)
                        hnb = onpool.tile([128, H], BF16, tag="hnb")
                        nc.scalar.activation(hnb[:], y2[:], ACTF.Identity,
                                             scale=rstd[:])
                        nc.sync.dma_start_transpose(hnT_t[t][:], hnb[:])
        P2.release()

        # ------------ ffn up (bf16, paired 512-col strips) (RIGHT) -----------
        with nc.named_scope("ffn_up"), \
             tc.tile_pool(name="upool", bufs=2, side="right") as upool, \
             tc.tile_pool(name="fpool", bufs=2, side="right") as fpool:
            for i in range(8):   # paired gate/val strips of 512
                wgv = upool.tile([128, 16, 1024], BF16, tag="wgv")
                nc.sync.dma_start(wgv[:, :, 0:512],
                                  wup_in[i].rearrange("p (k f) -> p k f", k=16))
                nc.sync.dma_start(wgv[:, :, 512:1024],
                                  wup_in[8 + i].rearrange("p (k f) -> p k f", k=16))
                for t in range(NT):
                    psg = mm_ps.tile([128, 512], F32, tag="ps_mm")
                    for kk in range(16):
                        nc.tensor.matmul(psg[:], hnT_t[t][:, kk, :],
                                         wgv[:, kk, 0:512],
                                         start=(kk == 0), stop=(kk == 15))
                    psv = upv_ps.tile([128, 512], F32, tag="ps_v")
                    for kk in range(16):
                        nc.tensor.matmul(psv[:], hnT_t[t][:, kk, :],
                                         wgv[:, kk, 512:1024],
                                         start=(kk == 0), stop=(kk == 15))
                    sgm = fpool.tile([128, 512], F32, tag="sgm")
                    nc.scalar.activation(sgm[:], psg[:], ACTF.Sigmoid)
                    sg = fpool.tile([128, 512], F32, tag="sg")
                    nc.vector.tensor_tensor(sg[:], sgm[:], psg[:], OP.mult)
                    av = fpool.tile([128, 512], F16, tag="av")
                    nc.vector.tensor_tensor(av[:], sg[:], psv[:], OP.mult)
                    nc.scalar.dma_start(act_d[t, :, i * 512:(i + 1) * 512], av[:])
                    nc.vector.tensor_reduce(amax_str[:, t, i:i + 1], av[:], AX.X,
                                            OP.max, apply_absolute_value=True)
        P3.release()

        # ------ act quant (PSUM magic, RIGHT) + ffn down (LEFT) --------
        actq_lo_t = [P4.tile([128, 16, 128], BF16, tag=f"p4_{t}", name=f"aql_{t}")
                     for t in range(NT)]
        actq_hi_t = [P1.tile([128, 16, 128], BF16, tag=f"p1_{t}", name=f"aqh_{t}")
                     for t in range(NT)]
        with nc.named_scope("ffn_dn"), \
             tc.tile_pool(name="aqpool", bufs=2) as aqpool, \
             tc.tile_pool(name="dpool", bufs=1) as dpool, \
             tc.tile_pool(name="dopool", bufs=3) as dopool:
            for t in range(NT):
                amr = aqpool.tile([128, 1], F32, tag="amra")
                nc.vector.tensor_reduce(amr[:], amax_str[:, t, :], AX.X, OP.max)
                amc = aqpool.tile([128, 1], F32, tag="amca")
                nc.vector.tensor_scalar_max(amc[:], amr[:], 1e-5)
                rec = aqpool.tile([128, 1], F32, tag="reca")
                nc.vector.reciprocal(rec[:], amc[:])
                nc.vector.tensor_scalar_mul(s_a[:, t:t + 1], rec[:], 127.0)
                nc.vector.tensor_scalar(dqact_all[:, t:t + 1], amc[:], dq_sb[:, 2:3],
                                        None, OP.mult)
                for half in range(2):
                    at2 = aqpool.tile([128, 2048], F16, tag="at2")
                    nc.sync.dma_start(at2[:], act_d[t, :, half * 2048:(half + 1) * 2048])
                    aq = aqpool.tile([128, 2048], BF16, tag="aqh")
                    for c in range(4):
                        mg = at_ps.tile([128, 512], F32, tag="ps_sc")
                        nc.scalar.activation(mg[:], at2[:, c * 512:(c + 1) * 512],
                                             ACTF.Identity, bias=magic_ap,
                                             scale=s_a[:, t:t + 1])
                        nc.vector.tensor_scalar_sub(aq[:, c * 512:(c + 1) * 512],
                                                    mg[:], float(MAGIC))
                    dst = actq_lo_t[t] if half == 0 else actq_hi_t[t]
                    nc.scalar.dma_start_transpose(dst[:], aq[:])
            for nn in range(4):   # 512-col output strips, double-buffered halves
                wh = [None, None]
                for half in range(2):
                    w = dpool.tile([128, 16, 512], BF16,
                                   tag=f"dw{(nn % 2) * 2 + half}")
                    nc.sync.dma_start(
                        w[:], wdn_in[nn, :, half * 8192:(half + 1) * 8192]
                        .rearrange("p (k f) -> p k f", k=16))
                    wh[half] = w
                for t in range(NT):
                    ps = mm_ps.tile([128, 512], F32, tag="ps_mm")
                    for kk in range(32):
                        src_ = actq_lo_t[t][:, kk, :] if kk < 16 \
                            else actq_hi_t[t][:, kk - 16, :]
                        nc.tensor.matmul(ps[:], src_, wh[kk // 16][:, kk % 16, :],
                                         start=(kk == 0), stop=(kk == 31))
                    tmp = dopool.tile([128, 512], F32, tag="d_tmp")
                    nc.vector.tensor_scalar_mul(tmp[:], ps[:], dqact_all[:, t:t + 1])
                    nc.gpsimd.dma_start(
                        out=tmp[:], in_=x1_d[t, :, nn * 512:(nn + 1) * 512],
                        accum_op=OP.add)
                    nc.sync.dma_start(out_d[t * 128:(t + 1) * 128,
                                            nn * 512:(nn + 1) * 512], tmp[:])
        upv_ps.release()
        cx_ps.release()
        at_ps.release()
        mm_ps.release()
        P4.release()
        P1.release()
        perm.release()

    nc.compile()
    return nc


_NC_CACHE = None


def _get_nc():
    global _NC_CACHE
    if _NC_CACHE is None:
        _NC_CACHE = build_program()
    return _NC_CACHE


def _ternarize(w):
    """Reference BitNet per-tensor ternary quant: returns (tern in {-1,0,1} f32,
    dqw = max(mean|w|, 1e-5))."""
    w = np.asarray(w, np.float32)
    dqw = np.float32(max(np.float32(np.mean(np.abs(w), dtype=np.float64)), np.float32(1e-5)))
    s = np.float32(1.0) / dqw
    tern = np.clip(np.round(w * s), -1.0, 1.0).astype(np.float32)
    return tern, dqw


def _wlayout(wt, nstrips, nk, ncol=512):
    """[K, N] (contraction-major) f32 -> [nstrips, 128, nk*ncol] bf16 SBUF layout."""
    K, N = wt.shape
    assert K == nk * 128 and N == nstrips * ncol
    r = wt.reshape(nk, 128, nstrips, ncol).transpose(2, 1, 0, 3).reshape(nstrips, 128, nk * ncol)
    return np.ascontiguousarray(r.astype(ml_dtypes.bfloat16))


def _host_inputs(x, attn_norm_w, ffn_norm_w, qkv_w, o_w, ffn_up_w, ffn_down_w):
    x = np.ascontiguousarray(np.asarray(x, np.float32))
    anw = np.ascontiguousarray(np.asarray(attn_norm_w, np.float32))
    fnw = np.ascontiguousarray(np.asarray(ffn_norm_w, np.float32))

    tern_qkv, dq_qkv = _ternarize(qkv_w)
    tern_o, dq_o = _ternarize(o_w)
    tern_dn, dq_dn = _ternarize(ffn_down_w)
    wqkv_r = _wlayout(tern_qkv.T, 24, 16, ncol=256)
    wo_r = _wlayout(tern_o.T, 4, 16)
    wdn_r = _wlayout(tern_dn.T, 4, 32)
    wup_r = _wlayout(np.asarray(ffn_up_w, np.float32).T, 16, 16)

    dqs = np.zeros((128, 4), np.float32)
    dqs[:, 0] = dq_qkv / 127.0
    dqs[:, 1] = dq_o / 127.0
    dqs[:, 2] = dq_dn / 127.0

    inv = 1.0 / (THETA ** (np.arange(0, HD, 2, dtype=np.float32) / HD))
    tpos = np.arange(S, dtype=np.float32)
    fr = np.outer(tpos, inv)                     # [S, 64]
    cos = np.cos(fr).astype(np.float32)
    sin = np.sin(fr).astype(np.float32)
    cos4 = np.tile(cos, (1, 4))                  # [S, 256]
    sin4 = np.tile(sin, (1, 4))

    in_maps = []
    for c in range(NCORES):
        b = c // 4
        t0 = (c % 4) * R
        in_maps.append({
            "x_sh": np.ascontiguousarray(x[b, t0:t0 + R, :]),
            "cos4_sh": np.ascontiguousarray(cos4[t0:t0 + R].reshape(NT, 128, 256)),
            "sin4_sh": np.ascontiguousarray(sin4[t0:t0 + R].reshape(NT, 128, 256)),
            "attn_norm_w": anw, "ffn_norm_w": fnw,
            "wqkv_r": wqkv_r, "wo_r": wo_r, "wup_r": wup_r, "wdn_r": wdn_r,
            "dqs": dqs,
        })
    return in_maps


def run(trace=False, **inputs):
    nc = _get_nc()
    in_maps = _host_inputs(**inputs)
    res = run_bass_kernel_spmd(nc, in_maps, list(range(NCORES)), trace=trace)
    out = np.empty((B, S, H), np.float32)
    for c in range(NCORES):
        b = c // 4
        t0 = (c % 4) * R
        out[b, t0:t0 + R, :] = res.results[c]["out_sh"]
    return out, res


def kernel(**inputs):
    out, _ = run(trace=False, **inputs)
    return out
